# revision 1
# baseline (speedup 1.0000x reference)
"""Causal self-attention (B=2, S=2048, H=16, D=64, HID=1024) on 8 TRN2 NeuronCores.

Sharding: core c handles batch b=c//4 and head group g=c%4 (4 heads = 256-wide
slice of the hidden dim). QKV + output projections are tensor-parallel over the
hid slice; attention is embarrassingly parallel over (b, h). Each core emits a
partial out^T [1024, 2048]; the host sums the 4 partials of each batch group,
transposes back, and adds the constant vector Wp@bv + bp (the value-bias and
output-bias fold into a single per-channel constant because attention rows sum
to 1).

Device layout is fully transposed (hid on partitions, tokens on the free dim)
so every matmul contracts along partitions. Scores are computed as
S^T[key, query] so the softmax numerator/denominator accumulate in PSUM across
key chunks; softmax uses exp without max subtraction (scores here are ~N(0,1),
so exp cannot overflow) and the denominator comes from an extra ones-column
appended to V. All matmuls run in float32r (1 cycle/row at moving-dim >= 256).

The attention loop is software-pipelined: the score matmuls for chunk-pair
i+1 are emitted before the exp/AV work of pair i, so the PE never waits the
ScalarE exp latency; output-projection matmuls for query-tile q are emitted
one head into query-tile q+1's stream to bridge the softmax-normalize gap.
"""

import numpy as np

import concourse.bass as bass
import concourse.mybir as mybir
import concourse.tile as tile
from concourse import bacc
from concourse.bass_utils import run_bass_kernel_spmd

B, S, H, D = 2, 2048, 16, 64
HID = H * D  # 1024
NCORES = 8
CPB = NCORES // B  # cores per batch group = 4
HPC = H // CPB  # heads per core = 4
ESL = HPC * D  # per-core hid slice = 256
KC = 128  # key chunk
QTS = 512  # query tile
NQT = S // QTS  # 4
NHC = HID // 128  # hid chunks = 8

F32 = mybir.dt.float32
F32R = mybir.dt.float32r
AF = mybir.ActivationFunctionType


def _emit(nc, tc, xT, wqT, wkT, wvT, wpT, bqk, msk, outT):
    from contextlib import ExitStack

    with ExitStack() as ctx:
        p_wv = ctx.enter_context(tc.tile_pool(name="pwv", bufs=8))
        p_wp = ctx.enter_context(tc.tile_pool(name="pwp", bufs=2))
        p_bm = ctx.enter_context(tc.tile_pool(name="pbm", bufs=1))
        p_qk = ctx.enter_context(tc.tile_pool(name="pqk", bufs=4))
        p_v = ctx.enter_context(tc.tile_pool(name="pv", bufs=16))
        p_yn = ctx.enter_context(tc.tile_pool(name="pyn", bufs=2))
        p_x = ctx.enter_context(tc.tile_pool(name="px", bufs=NHC))
        ps_mm = ctx.enter_context(tc.tile_pool(name="psmm", bufs=2, space="PSUM"))
        ps_s = ctx.enter_context(tc.tile_pool(name="pss", bufs=2, space="PSUM"))
        ps_y = ctx.enter_context(tc.tile_pool(name="psy", bufs=2, space="PSUM"))

        # Weight/bias/mask loads. DMA issue time (~0.6us per dma_start on the
        # issuing sequencer) gates the first QKV chains, so x pieces issue
        # from gpsimd while weights issue from sync.
        wv_sb = []
        for kc in range(NHC):
            t = p_wv.tile([128, ESL], F32R, tag="wv", name=f"wv{kc}")
            nc.sync.dma_start(t[:], wvT[bass.ts(kc, 128), :])
            wv_sb.append(t)
        bm = p_bm.tile([128, 4], F32, tag="bq", name="bm")
        nc.gpsimd.dma_start(bm[:], bqk[:, :])
        ones_sb = p_bm.tile([128, HPC], F32, tag="ones", name="ones_sb")
        nc.vector.memset(ones_sb[:], 1.0)
        msk_sb = p_bm.tile([128, 4 * QTS], F32, tag="msk", name="msk_sb")
        nc.gpsimd.dma_start(msk_sb[:], msk[:, :])
        wp_sb = []
        for ch in range(2):
            t = p_wp.tile([128, HID], F32R, tag="wp", name=f"wp{ch}")
            nc.sync.dma_start(t[:], wpT[bass.ts(ch, 128), :])
            wp_sb.append(t)

        # Persistent activation tiles
        QT_ = [p_qk.tile([128, S], F32R, tag="qk", name=f"QTt{i}") for i in range(2)]
        KT_ = [p_qk.tile([128, S], F32R, tag="qk", name=f"KTt{i}") for i in range(2)]
        V4 = [p_v.tile([128, HPC * 65], F32R, tag="v4", name=f"V4t{i}") for i in range(S // 128)]
        ynT = [p_yn.tile([128, S], F32R, tag="yn", name=f"ynTt{i}") for i in range(2)]

        x_sb = [p_x.tile([128, S], F32R, tag="xt", name=f"xt{kc}") for kc in range(NHC)]
        for st in range(NQT):
            for kc in range(NHC):
                nc.gpsimd.dma_start(
                    x_sb[kc][:, bass.ts(st, QTS)],
                    xT[bass.ts(kc, 128), bass.ts(st, QTS)],
                )

        def emit_vchain(st1):
            ps = ps_mm.tile([128, ESL], F32, tag="mm", name="vps_t")
            for kc in range(NHC):
                nc.tensor.matmul(
                    ps[:],
                    lhsT=x_sb[kc][:, bass.ts(st1, 128)],
                    rhs=wv_sb[kc][:],
                    start=(kc == 0),
                    stop=(kc == NHC - 1),
                )
            v3 = V4[st1][:].rearrange("p (h w) -> p h w", h=HPC)
            nc.vector.tensor_copy(v3[:, :, 0:64], ps[:].rearrange("p (h w) -> p h w", h=HPC))
            nc.vector.tensor_copy(
                v3[:, :, 64:65], ones_sb[:].rearrange("p (a b) -> p a b", b=1)
            )

        # Q/K weights stay resident: their projection chains for query
        # tiles 1-3 are woven into the attention stream as PE filler.
        p_wqk = ctx.enter_context(tc.tile_pool(name="pwqk", bufs=8))
        wq_sb, wk_sb = [], []
        for kc in range(NHC):
            for (lst, src, tag) in ((wq_sb, wqT, "wq"), (wk_sb, wkT, "wk")):
                t = p_wqk.tile([128, ESL], F32R, tag=tag, name=f"{tag}{kc}")
                nc.sync.dma_start(t[:], src[bass.ts(kc, 128), :])
                lst.append(t)

        def emit_qkchain(st, et, which):
            ssl = bass.ts(st, QTS)
            esl2 = bass.ts(et, 128)
            W, dst, bcol = (
                (wq_sb, QT_, et) if which == 0 else (wk_sb, KT_, 2 + et)
            )
            ps = ps_mm.tile([128, QTS], F32, tag="mm", name="ps_t")
            for kc in range(NHC):
                nc.tensor.matmul(
                    ps[:],
                    lhsT=W[kc][:, esl2],
                    rhs=x_sb[kc][:, ssl],
                    start=(kc == 0),
                    stop=(kc == NHC - 1),
                )
            nc.vector.tensor_scalar_add(dst[et][:, ssl], ps[:], bm[:, bcol : bcol + 1])

        # Prologue: the Q/K chains and V chunks query-tile 0 needs.
        for et in range(2):
            for which in range(2):
                emit_qkchain(0, et, which)
        for st1 in range(4):
            emit_vchain(st1)

        p_e = ctx.enter_context(tc.tile_pool(name="pe", bufs=2))
        p_r = ctx.enter_context(tc.tile_pool(name="pr", bufs=1))
        p_rb = ctx.enter_context(tc.tile_pool(name="prb", bufs=2))
        p_o = ctx.enter_context(tc.tile_pool(name="po", bufs=2))

        yps_cur = {}

        def emit_scores(qt_i, hh, cp):
            ch, h2 = hh // 2, hh % 2
            rows = slice(64 * h2, 64 * h2 + 64)
            qsl = bass.ts(qt_i, QTS)
            sps = ps_s.tile([128, 2 * QTS], F32, tag="sc", name="sps_t")
            for half in range(2):
                kci = 2 * cp + half
                nc.tensor.matmul(
                    sps[:, bass.ts(half, QTS)],
                    lhsT=KT_[ch][rows, bass.ts(kci, KC)],
                    rhs=QT_[ch][rows, qsl],
                    start=True,
                    stop=True,
                )
            return sps

        def emit_rest(qt_i, hh, cp, sps):
            ch, h2 = hh // 2, hh % 2
            ncp = 2 * qt_i + 2
            qsl = bass.ts(qt_i, QTS)
            if cp == 0:
                yps_cur[hh] = ps_y.tile([128, QTS], F32, tag="yps", name="yps_t")
            yps = yps_cur[hh]
            et_ = p_e.tile([128, 2 * QTS], F32R, tag="et", name="et_t")
            nc.scalar.activation(et_[:], sps[:], AF.Exp, scale=0.125)
            t2 = cp - 2 * qt_i
            if t2 >= 0:
                nc.vector.tensor_mul(et_[:], et_[:], msk_sb[:, bass.ts(t2, 2 * QTS)])
            for half in range(2):
                kci = 2 * cp + half
                nc.tensor.matmul(
                    yps[0:65, :],
                    lhsT=V4[kci][:, 65 * hh : 65 * hh + 65],
                    rhs=et_[:, bass.ts(half, QTS)],
                    start=(cp == 0 and half == 0),
                    stop=(cp == ncp - 1 and half == 1),
                )
            if cp == ncp - 1:
                s0 = p_r.tile([1, QTS], F32, tag="s0", name="s0_t")
                nc.vector.tensor_copy(s0[0:1, :], yps[64:65, :])
                rs = p_r.tile([1, QTS], F32, tag="rs", name="rs_t")
                nc.vector.reciprocal_approx_fast(rs[0:1, :], s0[0:1, :])
                rb = p_rb.tile([64, QTS], F32, tag="rb", name="rb_t")
                nc.gpsimd.partition_broadcast(rb[:], rs[0:1, :])
                nc.vector.tensor_mul(
                    ynT[ch][64 * h2 : 64 * h2 + 64, qsl], yps[0:64, :], rb[:]
                )

        def emit_proj_mt(qt_i, mt):
            qsl = bass.ts(qt_i, QTS)
            ops_ = ps_mm.tile([128, QTS], F32, tag="mm", name="ops_t")
            nc.tensor.matmul(
                ops_[:],
                lhsT=wp_sb[0][:, bass.ts(mt, 128)],
                rhs=ynT[0][:, qsl],
                start=True,
                stop=False,
            )
            nc.tensor.matmul(
                ops_[:],
                lhsT=wp_sb[1][:, bass.ts(mt, 128)],
                rhs=ynT[1][:, qsl],
                start=False,
                stop=True,
            )
            ot = p_o.tile([128, QTS], F32, tag="ot", name="ot_t")
            nc.vector.tensor_copy(ot[:], ops_[:])
            nc.sync.dma_start(outT[bass.ts(mt, 128), qsl], ot[:])

        # Global step sequence. Besides the softmax-pipelined attention steps,
        # each qtile's stream is padded with PE filler to keep the tensor
        # engine dense (HAM-warm) while ScalarE exp paces the softmax:
        #  - deferred V chains (chunks 4-7 during qtile 0, 8-11 during 1,
        #    12-13 during 2, 14-15 early in qtile 3),
        #  - output-projection chains of qtile q sprinkled into qtile q+2.
        fillers = {
            0: [("qkc", 1, et, w) for et in range(2) for w in range(2)]
            + [("vch", st1) for st1 in range(4, 8)],
            1: [("qkc", 2, et, w) for et in range(2) for w in range(2)]
            + [("vch", st1) for st1 in range(8, 12)],
            2: [("qkc", 3, et, w) for et in range(2) for w in range(2)]
            + [("vch", 12), ("vch", 13)]
            + [("proj", 0, mt) for mt in range(8)],
            3: [("vch", 14), ("vch", 15)]
            + [("proj", 1, mt) for mt in range(8)]
            + [("proj", 2, mt) for mt in range(8)],
        }
        seq = []
        for qt_i in range(NQT):
            ncp = 2 * qt_i + 2
            qsteps = []
            for hh in range(4):
                for cp in range(ncp):
                    qsteps.append(("att", qt_i, hh, cp))
            fl = fillers[qt_i]
            if qt_i == 3:
                head = fl[:2]
                rest = fl[2:]
                mixed = [qsteps[0], head[0], qsteps[1], head[1]] + qsteps[2:4]
                tail_steps = qsteps[4:]
                stride = max(1, len(tail_steps) // max(1, len(rest)))
                fi = 0
                for idx, s_ in enumerate(tail_steps):
                    mixed.append(s_)
                    if fi < len(rest) and (idx + 1) % stride == 0:
                        mixed.append(rest[fi])
                        fi += 1
                mixed.extend(rest[fi:])
                qsteps = mixed
            else:
                stride = max(1, len(qsteps) // max(1, len(fl)))
                mixed, fi = [], 0
                for idx, s_ in enumerate(qsteps):
                    mixed.append(s_)
                    if fi < len(fl) and (idx + 1) % stride == 0:
                        mixed.append(fl[fi])
                        fi += 1
                mixed.extend(fl[fi:])
                qsteps = mixed
            seq.extend(qsteps)
        for mt in range(HID // 128):
            seq.append(("proj", NQT - 1, mt))

        pend = None
        for s in seq:
            if s[0] == "att":
                _, qt_i, hh, cp = s
                sps = emit_scores(qt_i, hh, cp)
                if pend is not None:
                    emit_rest(*pend)
                pend = (qt_i, hh, cp, sps)
            elif s[0] == "vch":
                emit_vchain(s[1])
            elif s[0] == "qkc":
                emit_qkchain(s[1], s[2], s[3])
            else:
                _, pq, mt = s
                if pend is not None and pend[0] == pq:
                    emit_rest(*pend)
                    pend = None
                emit_proj_mt(pq, mt)
        if pend is not None:
            emit_rest(*pend)


def build():
    nc = bacc.Bacc("TRN2", target_bir_lowering=False, debug=False)
    xT = nc.dram_tensor("xT", [HID, S], F32R, kind="ExternalInput").ap()
    wqT = nc.dram_tensor("wqT", [HID, ESL], F32R, kind="ExternalInput").ap()
    wkT = nc.dram_tensor("wkT", [HID, ESL], F32R, kind="ExternalInput").ap()
    wvT = nc.dram_tensor("wvT", [HID, ESL], F32R, kind="ExternalInput").ap()
    wpT = nc.dram_tensor("wpT", [ESL, HID], F32R, kind="ExternalInput").ap()
    bqk = nc.dram_tensor("bqk", [128, 4], F32, kind="ExternalInput").ap()
    msk = nc.dram_tensor("msk", [128, 4 * QTS], F32, kind="ExternalInput").ap()
    outT = nc.dram_tensor("outT", [HID, S], F32, kind="ExternalOutput").ap()
    with tile.TileContext(nc) as tc:
        _emit(nc, tc, xT, wqT, wkT, wvT, wpT, bqk, msk, outT)
    nc.compile()
    return nc


_NC_CACHE = None


def _get_nc():
    global _NC_CACHE
    if _NC_CACHE is None:
        _NC_CACHE = build()
    return _NC_CACHE


def _mask_np():
    m = np.zeros((128, 4 * QTS), np.float32)
    r = np.arange(128)[:, None]
    c = np.arange(QTS)[None, :]
    for t in range(4):
        m[:, QTS * t : QTS * (t + 1)] = (c >= 128 * t + r).astype(np.float32)
    return m


def make_in_maps(x, Wq, bq, Wk, bk, Wv, bv, Wp, bp):
    msk = _mask_np()
    in_maps = []
    for c in range(NCORES):
        b, g = c // CPB, c % CPB
        es = slice(ESL * g, ESL * (g + 1))
        bqk = np.stack(
            [bq[es][:128], bq[es][128:], bk[es][:128], bk[es][128:]], axis=1
        ).astype(np.float32)
        in_maps.append(
            {
                "xT": np.ascontiguousarray(x[b].T),
                "wqT": np.ascontiguousarray(Wq[es].T),
                "wkT": np.ascontiguousarray(Wk[es].T),
                "wvT": np.ascontiguousarray(Wv[es].T),
                "wpT": np.ascontiguousarray(Wp[:, es].T),
                "bqk": np.ascontiguousarray(bqk),
                "msk": msk,
            }
        )
    return in_maps


def gather_output(results, Wp, bv, bp):
    cvec = (Wp @ bv + bp).astype(np.float32)
    out = np.empty((B, S, HID), np.float32)
    for b in range(B):
        acc = np.zeros((HID, S), np.float32)
        for g in range(CPB):
            acc += results[b * CPB + g]["outT"]
        out[b] = acc.T + cvec[None, :]
    return out


def kernel(x, Wq, bq, Wk, bk, Wv, bv, Wp, bp):
    x = np.asarray(x, np.float32)
    nc = _get_nc()
    in_maps = make_in_maps(x, Wq, bq, Wk, bk, Wv, bv, Wp, bp)
    res = run_bass_kernel_spmd(nc, in_maps, core_ids=list(range(NCORES)))
    return gather_output(res.results, np.asarray(Wp), np.asarray(bv), np.asarray(bp))



# revision 3
# speedup vs baseline: 1.2340x; 1.2340x over previous
"""Causal self-attention (B=2, S=2048, H=16, D=64, HID=1024) on 8 TRN2 NeuronCores.

Sharding: core c handles batch b=c//4 and head group g=c%4 (4 heads = 256-wide
slice of the hidden dim). QKV + output projections are tensor-parallel over the
hid slice; attention is embarrassingly parallel over (b, h). Each core emits a
partial out^T; the host sums the 4 partials of each batch group, transposes
back, and adds the constant vector Wp@bv + bp (value-bias and output-bias fold
into one per-channel constant because attention rows sum to 1).

v2 over the fp32r baseline:
- All matmul operands are bf16 (activations, weights, exp-scores). PSUM stays
  fp32. This halves HBM traffic, enables FWL fast weight loads (bf16-only),
  and unlocks 2x DVE modes for the mask multiplies.
- Host pre-packs every input into its exact SBUF layout so the whole input
  load is 8 large descriptors (>=0.5 MiB each) spread over 4 engine queues,
  instead of 58 small ones: kills the ~31us issue-serialized startup stall
  that kept HAM cold until 53us.
- Score matmuls are row-packed: the two heads of a 128-row K/Q tile use
  disjoint PE row groups (base_partition 0 / 64), so the pair runs
  concurrently -- scores cost ~1 matmul slot instead of 2.
- Output is staged in bf16 and stored with one 1 MiB DMA per query tile.

Softmax uses exp without max subtraction (scores ~N(0,1)); the denominator
comes from a ones-column appended to V (65-wide AV stationary).
"""

import numpy as np

import concourse.bass as bass
import concourse.mybir as mybir
import concourse.tile as tile
from concourse import bacc
from concourse.bass_utils import run_bass_kernel_spmd

B, S, H, D = 2, 2048, 16, 64
HID = H * D  # 1024
NCORES = 8
CPB = NCORES // B  # cores per batch group = 4
HPC = H // CPB  # heads per core = 4
ESL = HPC * D  # per-core hid slice = 256
KC = 128  # key chunk
QTS = 512  # query tile
NQT = S // QTS  # 4
NHC = HID // 128  # hid chunks = 8

F32 = mybir.dt.float32
BF16 = mybir.dt.bfloat16
AF = mybir.ActivationFunctionType


def _emit(nc, tc, xP, wQK, wVP, bqk, msk, outP):
    from contextlib import ExitStack

    with ExitStack() as ctx:
        p_w = ctx.enter_context(tc.tile_pool(name="pw", bufs=2))
        p_bm = ctx.enter_context(tc.tile_pool(name="pbm", bufs=1))
        p_qk = ctx.enter_context(tc.tile_pool(name="pqk", bufs=4))
        p_v = ctx.enter_context(tc.tile_pool(name="pv", bufs=16))
        p_yn = ctx.enter_context(tc.tile_pool(name="pyn", bufs=2))
        p_x = ctx.enter_context(tc.tile_pool(name="px", bufs=1))
        ps_mm = ctx.enter_context(tc.tile_pool(name="psmm", bufs=1, space="PSUM"))
        ps_s = ctx.enter_context(tc.tile_pool(name="pss", bufs=2, space="PSUM"))
        ps_y = ctx.enter_context(tc.tile_pool(name="psy", bufs=3, space="PSUM"))

        # ---- bulk input loads: one big DMA per tensor, 4 queues ----
        x_sb = p_x.tile([128, 4 * 4096], BF16, tag="xt", name="x_sb")
        for st in range(NQT):
            nc.gpsimd.dma_start(x_sb[:, bass.ts(st, 4096)], xP[st, :, :])
        wqk_sb = p_w.tile([128, 4096], BF16, tag="wqk", name="wqk_sb")
        nc.sync.dma_start(wqk_sb[:], wQK[:, :])
        wvp_sb = p_w.tile([128, 4096], BF16, tag="wvp", name="wvp_sb")
        nc.scalar.dma_start(wvp_sb[:], wVP[:, :])
        bm = p_bm.tile([128, 4], F32, tag="bq", name="bm")
        nc.scalar.dma_start(bm[:], bqk[:, :])
        msk_sb = p_bm.tile([128, 4 * QTS], BF16, tag="msk", name="msk_sb")
        nc.scalar.dma_start(msk_sb[:], msk[:, :])
        ones_sb = p_bm.tile([128, HPC], BF16, tag="ones", name="ones_sb")
        nc.vector.memset(ones_sb[:], 1.0)

        def x_sl(kc, c0, w):
            st = c0 // QTS
            return x_sb[:, st * 4096 + kc * 512 + (c0 - st * QTS) : st * 4096 + kc * 512 + (c0 - st * QTS) + w]

        def wq_sl(kc, et):
            return wqk_sb[:, kc * 256 + et * 128 : kc * 256 + et * 128 + 128]

        def wk_sl(kc, et):
            return wqk_sb[:, 2048 + kc * 256 + et * 128 : 2048 + kc * 256 + et * 128 + 128]

        def wv_sl(kc):
            return wvp_sb[:, kc * 256 : kc * 256 + 256]

        def wp_sl(ch, mt):
            return wvp_sb[:, 2048 + ch * 1024 + mt * 128 : 2048 + ch * 1024 + mt * 128 + 128]

        # Persistent activation tiles (bf16)
        QT_ = [p_qk.tile([128, S], BF16, tag="qk", name=f"QTt{i}") for i in range(2)]
        KT_ = [p_qk.tile([128, S], BF16, tag="qk", name=f"KTt{i}") for i in range(2)]
        V4 = [p_v.tile([128, HPC * 65], BF16, tag="v4", name=f"V4t{i}") for i in range(S // 128)]
        ynT = [p_yn.tile([128, S], BF16, tag="yn", name=f"ynTt{i}") for i in range(2)]

        def emit_vchain(st1):
            ps = ps_mm.tile([128, 512], F32, tag="mm", name="vps_t")
            for kc in range(NHC):
                nc.tensor.matmul(
                    ps[:, 0:ESL],
                    lhsT=x_sl(kc, st1 * 128, 128),
                    rhs=wv_sl(kc),
                    start=(kc == 0),
                    stop=(kc == NHC - 1),
                )
            v3 = V4[st1][:].rearrange("p (h w) -> p h w", h=HPC)
            nc.vector.tensor_copy(
                v3[:, :, 0:64], ps[:, 0:ESL].rearrange("p (h w) -> p h w", h=HPC)
            )
            nc.vector.tensor_copy(
                v3[:, :, 64:65], ones_sb[:].rearrange("p (a b) -> p a b", b=1)
            )

        def emit_qkchain(st, et, which):
            W, dst, bcol = (wq_sl, QT_, et) if which == 0 else (wk_sl, KT_, 2 + et)
            ps = ps_mm.tile([128, 512], F32, tag="mm", name="ps_t")
            for kc in range(NHC):
                nc.tensor.matmul(
                    ps[:],
                    lhsT=W(kc, et),
                    rhs=x_sl(kc, st * QTS, QTS),
                    start=(kc == 0),
                    stop=(kc == NHC - 1),
                )
            nc.vector.tensor_scalar_add(
                dst[et][:, bass.ts(st, QTS)], ps[:], bm[:, bcol : bcol + 1]
            )

        # Prologue: qt0's Q/K chains and V chunks.
        for et in range(2):
            for which in range(2):
                emit_qkchain(0, et, which)
        for st1 in range(4):
            emit_vchain(st1)

        p_e = ctx.enter_context(tc.tile_pool(name="pe", bufs=2))
        p_r = ctx.enter_context(tc.tile_pool(name="pr", bufs=1))
        p_rb = ctx.enter_context(tc.tile_pool(name="prb", bufs=2))
        p_o = ctx.enter_context(tc.tile_pool(name="po", bufs=2))

        yps_cur = {}

        def emit_scores(qt_i, ch, kci):
            # Row-packed pair: head h2=0 on PE rows 0-63, h2=1 on rows 64-127.
            qsl = bass.ts(qt_i, QTS)
            ksl = bass.ts(kci, KC)
            sps = ps_s.tile([128, 2 * QTS], F32, tag="sc", name="sps_t")
            for h2 in range(2):
                rows = slice(64 * h2, 64 * h2 + 64)
                nc.tensor.matmul(
                    sps[:, bass.ts(h2, QTS)],
                    lhsT=KT_[ch][rows, ksl],
                    rhs=QT_[ch][rows, qsl],
                    start=True,
                    stop=True,
                )
            return sps

        def emit_rest(qt_i, ch, kci, sps):
            nck = 4 * qt_i + 4
            qsl = bass.ts(qt_i, QTS)
            if kci == 0:
                yps_cur[2 * ch] = ps_y.tile([128, QTS], F32, tag="yps", name="yps_t")
                yps_cur[2 * ch + 1] = ps_y.tile([128, QTS], F32, tag="yps", name="yps_t")
            et_ = p_e.tile([128, 2 * QTS], BF16, tag="et", name="et_t")
            nc.scalar.activation(et_[:], sps[:], AF.Exp, scale=0.125)
            t2 = kci - 4 * qt_i
            if t2 >= 0:
                for h2 in range(2):
                    nc.vector.tensor_mul(
                        et_[:, bass.ts(h2, QTS)],
                        et_[:, bass.ts(h2, QTS)],
                        msk_sb[:, bass.ts(t2, QTS)],
                    )
            for h2 in range(2):
                hh = 2 * ch + h2
                yps = yps_cur[hh]
                nc.tensor.matmul(
                    yps[0:65, :],
                    lhsT=V4[kci][:, 65 * hh : 65 * hh + 65],
                    rhs=et_[:, bass.ts(h2, QTS)],
                    start=(kci == 0),
                    stop=(kci == nck - 1),
                )
            if kci == nck - 1:
                for h2 in range(2):
                    hh = 2 * ch + h2
                    yps = yps_cur[hh]
                    s0 = p_r.tile([1, QTS], F32, tag="s0", name="s0_t")
                    nc.vector.tensor_copy(s0[0:1, :], yps[64:65, :])
                    rs = p_r.tile([1, QTS], F32, tag="rs", name="rs_t")
                    nc.vector.reciprocal_approx_fast(rs[0:1, :], s0[0:1, :])
                    rb = p_rb.tile([64, QTS], F32, tag="rb", name="rb_t")
                    nc.gpsimd.partition_broadcast(rb[:], rs[0:1, :])
                    nc.vector.tensor_mul(
                        ynT[ch][64 * h2 : 64 * h2 + 64, qsl], yps[0:64, :], rb[:]
                    )

        ot_cur = {}

        def emit_proj_mt(qt_i, mt):
            qsl = bass.ts(qt_i, QTS)
            if mt == 0:
                ot_cur[qt_i] = p_o.tile([128, 8 * QTS], BF16, tag="ot", name="ot_t")
            ops_ = ps_mm.tile([128, QTS], F32, tag="mm", name="ops_t")
            nc.tensor.matmul(
                ops_[:], lhsT=wp_sl(0, mt), rhs=ynT[0][:, qsl], start=True, stop=False
            )
            nc.tensor.matmul(
                ops_[:], lhsT=wp_sl(1, mt), rhs=ynT[1][:, qsl], start=False, stop=True
            )
            nc.vector.tensor_copy(ot_cur[qt_i][:, bass.ts(mt, QTS)], ops_[:])
            if mt == 7:
                src = ot_cur[qt_i][:].rearrange("p (m c) -> p m c", m=8)
                nc.sync.dma_start(
                    outP[:, :, bass.ts(qt_i, QTS)], src
                )

        # Step schedule: attention steps with PE-filler (deferred chains and
        # output projections) mixed in to bridge softmax latency.
        fillers = {
            0: [("qkc", 1, et, w) for et in range(2) for w in range(2)]
            + [("vch", st1) for st1 in range(4, 8)],
            1: [("qkc", 2, et, w) for et in range(2) for w in range(2)]
            + [("vch", st1) for st1 in range(8, 12)],
            2: [("qkc", 3, et, w) for et in range(2) for w in range(2)]
            + [("vch", 12), ("vch", 13)]
            + [("proj", 0, mt) for mt in range(8)],
            3: [("vch", 14), ("vch", 15)]
            + [("proj", 1, mt) for mt in range(8)]
            + [("proj", 2, mt) for mt in range(8)],
        }
        seq = []
        for qt_i in range(NQT):
            nck = 4 * qt_i + 4
            qsteps = []
            for ch in range(2):
                for kci in range(nck):
                    qsteps.append(("att", qt_i, ch, kci))
            fl = fillers[qt_i]
            stride = max(1, len(qsteps) // max(1, len(fl)))
            mixed, fi = [], 0
            for idx, s_ in enumerate(qsteps):
                mixed.append(s_)
                if fi < len(fl) and (idx + 1) % stride == 0:
                    mixed.append(fl[fi])
                    fi += 1
            mixed.extend(fl[fi:])
            seq.extend(mixed)
        for mt in range(HID // 128):
            seq.append(("proj", NQT - 1, mt))

        pend = None
        for s in seq:
            if s[0] == "att":
                _, qt_i, ch, kci = s
                sps = emit_scores(qt_i, ch, kci)
                if pend is not None:
                    emit_rest(*pend)
                pend = (qt_i, ch, kci, sps)
            elif s[0] == "vch":
                emit_vchain(s[1])
            elif s[0] == "qkc":
                emit_qkchain(s[1], s[2], s[3])
            else:
                _, pq, mt = s
                if pend is not None and pend[0] == pq:
                    emit_rest(*pend)
                    pend = None
                emit_proj_mt(pq, mt)
        if pend is not None:
            emit_rest(*pend)


def build():
    nc = bacc.Bacc("TRN2", target_bir_lowering=False, debug=False)
    xP = nc.dram_tensor("xP", [4, 128, 4096], BF16, kind="ExternalInput").ap()
    wQK = nc.dram_tensor("wQK", [128, 4096], BF16, kind="ExternalInput").ap()
    wVP = nc.dram_tensor("wVP", [128, 4096], BF16, kind="ExternalInput").ap()
    bqk = nc.dram_tensor("bqk", [128, 4], F32, kind="ExternalInput").ap()
    msk = nc.dram_tensor("msk", [128, 4 * QTS], BF16, kind="ExternalInput").ap()
    outP = nc.dram_tensor("outP", [128, 8, S], BF16, kind="ExternalOutput").ap()
    with tile.TileContext(nc) as tc:
        _emit(nc, tc, xP, wQK, wVP, bqk, msk, outP)
    nc.compile()
    return nc


_NC_CACHE = None


def _get_nc():
    global _NC_CACHE
    if _NC_CACHE is None:
        _NC_CACHE = build()
    return _NC_CACHE


def _mask_np():
    m = np.zeros((128, 4 * QTS), np.float32)
    r = np.arange(128)[:, None]
    c = np.arange(QTS)[None, :]
    for t in range(4):
        m[:, QTS * t : QTS * (t + 1)] = (c >= 128 * t + r).astype(np.float32)
    return m


def make_in_maps(x, Wq, bq, Wk, bk, Wv, bv, Wp, bp):
    import ml_dtypes

    bf = ml_dtypes.bfloat16
    msk = _mask_np().astype(bf)
    in_maps = []
    for c in range(NCORES):
        b, g = c // CPB, c % CPB
        es = slice(ESL * g, ESL * (g + 1))
        bqk = np.stack(
            [bq[es][:128], bq[es][128:], bk[es][:128], bk[es][128:]], axis=1
        ).astype(np.float32)
        # xP[st, p, kc*512+c] = x[b, st*512+c, kc*128+p]
        xt = np.asarray(x[b], np.float32).T  # [1024, 2048]
        xP = (
            xt.reshape(8, 128, 4, 512).transpose(2, 1, 0, 3).reshape(4, 128, 4096)
        ).astype(bf)

        def packw(WT):  # WT [1024, 256] -> [128, 2048]: [p, kc*256+e]
            return WT.reshape(8, 128, 256).transpose(1, 0, 2).reshape(128, 2048)

        wq = packw(np.ascontiguousarray(Wq[es].T).astype(np.float32))
        wk = packw(np.ascontiguousarray(Wk[es].T).astype(np.float32))
        wv = packw(np.ascontiguousarray(Wv[es].T).astype(np.float32))
        # wp [256, 1024] -> [128, ch*1024+c]
        wpT = np.ascontiguousarray(Wp[:, es].T).astype(np.float32)
        wp = wpT.reshape(2, 128, 1024).transpose(1, 0, 2).reshape(128, 2048)
        in_maps.append(
            {
                "xP": xP,
                "wQK": np.ascontiguousarray(np.concatenate([wq, wk], 1)).astype(bf),
                "wVP": np.ascontiguousarray(np.concatenate([wv, wp], 1)).astype(bf),
                "bqk": np.ascontiguousarray(bqk),
                "msk": msk,
            }
        )
    return in_maps


def gather_output(results, Wp, bv, bp):
    cvec = (Wp @ bv + bp).astype(np.float32)
    out = np.empty((B, S, HID), np.float32)
    for b in range(B):
        acc = np.zeros((HID, S), np.float32)
        for g in range(CPB):
            o = np.asarray(results[b * CPB + g]["outP"], np.float32)  # [128, 8, 2048]
            acc += o.transpose(1, 0, 2).reshape(HID, S)
        out[b] = acc.T + cvec[None, :]
    return out


def kernel(x, Wq, bq, Wk, bk, Wv, bv, Wp, bp):
    x = np.asarray(x, np.float32)
    nc = _get_nc()
    in_maps = make_in_maps(x, Wq, bq, Wk, bk, Wv, bv, Wp, bp)
    res = run_bass_kernel_spmd(nc, in_maps, core_ids=list(range(NCORES)))
    return gather_output(res.results, np.asarray(Wp), np.asarray(bv), np.asarray(bp))
